# revision 9
# baseline (speedup 1.0000x reference)
"""Top-1 MoE (8 experts) expert-parallel kernel for Trainium2, 8 NeuronCores.

Strategy:
  - Host: argmax(router_logits) -> per-token expert id; tokens are grouped by
    expert and packed into per-core bins (the "all-to-all dispatch" happens
    host-side since we receive full inputs and return full outputs).
  - Load balance: every core runs the same program with `nsegs` token
    segments of fixed sizes (S1..Sk); each segment has its own weight slot
    (an expert id, per-core data). A DP assigns experts to the 8*nsegs bins
    so per-core capacity C = sum(sizes) is minimized (~1.6% above the
    perfect T/8 balance instead of the 23% a one-expert-per-core split
    costs with skewed routing).
  - Device (SPMD): dense 2-GEMM SiLU MLP in bf16 with fp32 PSUM
    accumulation, one weight stream per segment.
  - Host: scatter per-core outputs back to token order ("combine").

Per-core problem per segment s: x[S_s, D] @ w1[slot_s].T -> silu ->
@ w2[slot_s].T, with D=2048, F=4096.

Device layouts (partition-major so every DMA is a plain slice):
  xt  [128, 16, C]     bf16   xt[p, ko, t]     = x[t, ko*128+p]
  w1t [S, 128, 16, F]  bf16   w1t[s, p, ko, f] = w1[slot_s][f, ko*128+p]
  w2t [S, 128, 32, D]  bf16   w2t[s, p, ko, d] = w2[slot_s][d, ko*128+p]
  yt  [128, 16, C]     f32    yt[p, do, t]     = y[t, do*128+p]
"""

import numpy as np
import ml_dtypes

BF16 = ml_dtypes.bfloat16

P = 128
D = 2048
F = 4096
E = 8
N_CORES = 8
TCHUNK = 512  # max token chunk = matmul free dim (one PSUM bank of fp32)
W1B = 512     # GEMM1 weight block width (columns of F per streamed tile)
W2B = 256     # GEMM2 weight block width (columns of D per streamed tile)

KO1 = D // P  # 16 contraction tiles for GEMM1
KO2 = F // P  # 32 contraction tiles for GEMM2

_BUILD_CACHE = {}


def dedupe_ldweights(nc):
    """Delete InstLdweights that reload the stationary tile just loaded
    (identical source AP, no intervening PE-array clobber). The following
    non-self-loading InstMatmult then reuses the already-loaded weights.
    Only sync-free ldweights are removed, so all semaphore waits/updates
    are preserved. Validated bit-exact in CoreSim."""
    import concourse.mybir as mybir
    ndel = 0
    for fn in nc.m.functions:
        for blk in fn.blocks:
            last_key = None
            keep = []
            for inst in blk.instructions:
                tn = type(inst).__name__
                if tn == "InstLdweights":
                    si = inst.sync_info
                    has_sync = si is not None and (
                        len(si.on_wait) > 0 or len(si.on_update) > 0
                    )
                    key = str(inst.ins[0])
                    if (not has_sync) and key == last_key:
                        ndel += 1
                        continue
                    last_key = key
                elif tn == "InstMatmult":
                    pass  # consumes loaded weights, no clobber
                elif getattr(inst, "engine", None) == mybir.EngineType.PE:
                    last_key = None  # other PE instruction: conservative reset
                keep.append(inst)
            blk.instructions[:] = keep
    return ndel


def _chunks_of(size, base):
    """Split a segment into near-equal chunks of width <= TCHUNK."""
    n = -(-size // TCHUNK)
    out = []
    t0 = 0
    for i in range(n):
        w = (size - t0 + (n - i) - 1) // (n - i)
        out.append((base + t0, w))
        t0 += w
    return out


def build_nc(sizes, act="silu", loop_reps=None):
    """Build + compile the per-core Bass program for segment sizes `sizes`.

    Each segment uses weight slot s of the w1t/w2t inputs. loop_reps wraps
    one pass in a hardware For_i loop (for slope-based HW timing); results
    are identical since the computation is idempotent.
    """
    sizes = tuple(int(s) for s in sizes)
    key = (sizes, act, loop_reps)
    if key in _BUILD_CACHE:
        return _BUILD_CACHE[key]

    import concourse.bacc as bacc
    import concourse.mybir as mybir
    from concourse import tile

    S = len(sizes)
    C = sum(sizes)
    dt = mybir.dt
    act_fn = {
        "silu": mybir.ActivationFunctionType.Silu,
        "sigmoid": mybir.ActivationFunctionType.Sigmoid,
    }[act]
    nc = bacc.Bacc("TRN2", target_bir_lowering=False, debug=False)

    xt_d = nc.dram_tensor("xt", [P, KO1, C], dt.bfloat16, kind="ExternalInput")
    if S == 1:
        w1t_d = nc.dram_tensor("w1t", [P, KO1, F], dt.bfloat16, kind="ExternalInput")
        w2t_d = nc.dram_tensor("w2t", [P, KO2, D], dt.bfloat16, kind="ExternalInput")
        w1s = lambda s: w1t_d
        w2s = lambda s: w2t_d
    else:
        w1t_d = nc.dram_tensor("w1t", [S, P, KO1, F], dt.bfloat16, kind="ExternalInput")
        w2t_d = nc.dram_tensor("w2t", [S, P, KO2, D], dt.bfloat16, kind="ExternalInput")
        w1s = lambda s: w1t_d[s]
        w2s = lambda s: w2t_d[s]
    yt_d = nc.dram_tensor("yt", [P, KO1, C], dt.float32, kind="ExternalOutput")

    seg_chunks = []  # per segment: list of (t0, tw)
    base = 0
    for s in sizes:
        seg_chunks.append(_chunks_of(s, base))
        base += s

    N1 = F // W1B
    N2 = D // W2B

    with tile.TileContext(nc) as tc:
        with (
            tc.tile_pool(name="xpool", bufs=1) as xpool,
            tc.tile_pool(name="hpool", bufs=1) as hpool,
            tc.tile_pool(name="wpool", bufs=4) as wpool,
            tc.tile_pool(name="ypool", bufs=4) as ypool,
            tc.tile_pool(name="cpool", bufs=1) as cpool,
            tc.tile_pool(name="pspool", bufs=8, space="PSUM") as pspool,
        ):
            zbias = cpool.tile([P, 1], dt.float32)
            nc.any.memset(zbias[:], 0.0)

            x_sb = xpool.tile([P, KO1, C], dt.bfloat16)
            h_sb = hpool.tile([P, KO2, C], dt.bfloat16)

            # Load x by chunk so GEMM1 can start after the first chunk.
            # SWDGE (gpsimd) path: x never queues behind the weight prefetch
            # on the SP HWDGE ring, so the first matmul starts sooner.
            for chunks in seg_chunks:
                for (t0, tw) in chunks:
                    nc.gpsimd.dma_start(
                        x_sb[:, :, t0 : t0 + tw], xt_d[:, :, t0 : t0 + tw]
                    )

            def one_pass(rep):
                # chunk-outer / k-inner: 16-32 consecutive matmuls accumulate
                # into ONE PSUM bank. Measured faster than k-outer/chunk-inner
                # (which saves 1024 LDWEIGHTS via dedupe but pays a per-matmul
                # PSUM bank-switch: 580us vs 564us on HW).
                # GEMM1 + SiLU: h[f, t] = silu(sum_d w1t[d, f] * x[d, t])
                for si in range(S):
                    chunks = seg_chunks[si]
                    for mb in range(N1):
                        w1_sb = wpool.tile(
                            [P, KO1, W1B], dt.bfloat16, tag="w",
                            name=f"w1_{rep}_{si}_{mb}",
                        )
                        nc.sync.dma_start(
                            w1_sb[:], w1s(si)[:, :, mb * W1B : (mb + 1) * W1B]
                        )
                        for ms in range(W1B // P):
                            fo = mb * (W1B // P) + ms
                            for ci, (t0, tw) in enumerate(chunks):
                                ps = pspool.tile(
                                    [P, TCHUNK], dt.float32, tag="ps",
                                    name=f"ps1_{rep}_{si}_{mb}_{ms}_{ci}",
                                )
                                for k in range(KO1):
                                    nc.tensor.matmul(
                                        ps[:, :tw],
                                        w1_sb[:, k, ms * P : (ms + 1) * P],
                                        x_sb[:, k, t0 : t0 + tw],
                                        start=(k == 0),
                                        stop=(k == KO1 - 1),
                                    )
                                nc.scalar.activation(
                                    h_sb[:, fo, t0 : t0 + tw],
                                    ps[:, :tw],
                                    act_fn,
                                    bias=zbias[:],
                                )

                # GEMM2: y[d, t] = sum_f w2t[f, d] * h[f, t]
                for si in range(S):
                    chunks = seg_chunks[si]
                    for db in range(N2):
                        w2_sb = wpool.tile(
                            [P, KO2, W2B], dt.bfloat16, tag="w",
                            name=f"w2_{rep}_{si}_{db}",
                        )
                        nc.sync.dma_start(
                            w2_sb[:], w2s(si)[:, :, db * W2B : (db + 1) * W2B]
                        )
                        for ds in range(W2B // P):
                            do = db * (W2B // P) + ds
                            for ci, (t0, tw) in enumerate(chunks):
                                ps = pspool.tile(
                                    [P, TCHUNK], dt.float32, tag="ps",
                                    name=f"ps2_{rep}_{si}_{db}_{ds}_{ci}",
                                )
                                for k in range(KO2):
                                    nc.tensor.matmul(
                                        ps[:, :tw],
                                        w2_sb[:, k, ds * P : (ds + 1) * P],
                                        h_sb[:, k, t0 : t0 + tw],
                                        start=(k == 0),
                                        stop=(k == KO2 - 1),
                                    )
                                y_sb = ypool.tile(
                                    [P, TCHUNK], dt.float32, tag="y",
                                    name=f"y_{rep}_{si}_{db}_{ds}_{ci}",
                                )
                                nc.vector.tensor_copy(y_sb[:, :tw], ps[:, :tw])
                                # y stores go through the ACT HWDGE ring so
                                # they never queue ahead of weight prefetch on
                                # the SP ring (HWDGE is FIFO per engine).
                                nc.scalar.dma_start(
                                    yt_d[:, do, t0 : t0 + tw], y_sb[:, :tw]
                                )

            if loop_reps is not None and loop_reps > 1:
                with tc.For_i(0, loop_reps, 1):
                    one_pass(0)
            else:
                one_pass(0)

    nc.compile()
    dedupe_ldweights(nc)
    _BUILD_CACHE[key] = nc
    return nc


# ---------------------------------------------------------------- balancing

def _min_cover_opts(c, sizes, max_bins):
    """All minimal k-vectors with sum_j k_j*S_j >= c."""
    import itertools
    if c == 0:
        return [tuple([0] * len(sizes))]
    opts = []
    maxk = [min(max_bins, -(-c // s)) for s in sizes]
    for ks in itertools.product(*[range(k + 1) for k in maxk]):
        cap = sum(k * s for k, s in zip(ks, sizes))
        if cap < c:
            continue
        if any(k > 0 and cap - s >= c for k, s in zip(ks, sizes)):
            continue
        opts.append(ks)
    return opts


def _solve_alloc(counts, sizes, nbins=N_CORES):
    """DP: assign each expert k_j bins of size S_j (sum_j k_j*S_j >= count),
    using at most `nbins` bins of each size overall. Returns per-expert
    k-vectors or None."""
    ns = len(sizes)
    levels = [{tuple([0] * ns): None}]
    for c in counts:
        opts = _min_cover_opts(c, sizes, nbins)
        new = {}
        for st in levels[-1]:
            for ks in opts:
                nst = tuple(u + k for u, k in zip(st, ks))
                if all(u <= nbins for u in nst) and nst not in new:
                    new[nst] = (st, ks)
        if not new:
            return None
        levels.append(new)
    state = next(iter(levels[-1]))
    alloc = []
    for lev in range(len(counts), 0, -1):
        prev, ks = levels[lev][state]
        alloc.append(ks)
        state = prev
    alloc.reverse()
    return alloc


def _pass_cost(sizes):
    """Relative per-pass cost model (cycles): streamed rows + per-segment
    weight restream overhead + per-chunk overhead. Narrow chunks stream
    slightly worse on HW (measured ~5% at width 128), so penalize them."""
    C = sum(sizes)
    nseg = len(sizes)
    cost = 1024 * C + 24576 * nseg
    for s in sizes:
        for _, w in _chunks_of(s, 0):
            cost += 8192 + 24 * max(0, 224 - w)
    return cost


def choose_scheme(counts):
    """Pick segment sizes minimizing _pass_cost subject to DP feasibility."""
    counts = list(counts)
    best = (float("inf"), (max(max(counts), P),), None)

    # 1 segment
    s1 = max(max(counts), P)
    alloc = _solve_alloc(counts, (s1,))
    if alloc is not None:
        best = min(best, (_pass_cost((s1,)), (s1,), alloc))

    # 2 segments: S1 >= S2, coarse then fine grid
    def try_sizes(sizes):
        nonlocal best
        if sizes[-1] < 32:
            return
        cost = _pass_cost(sizes)
        if cost >= best[0]:
            return
        alloc = _solve_alloc(counts, sizes)
        if alloc is not None:
            best = min(best, (cost, sizes, alloc))

    Tmax = max(max(counts), -(-sum(counts) // N_CORES))
    for C in range(-(-sum(counts) // (N_CORES * 16)) * 16, Tmax + 257, 16):
        for s2 in range(32, C // 2 + 1, 16):
            try_sizes((C - s2, s2))
        if best[2] is not None and best[0] <= 1024 * (C + 64) + 2 * 24576:
            break

    # 3 segments (s1-major grid so 512-wide bulk segments are explored)
    for C in range(-(-sum(counts) // (N_CORES * 16)) * 16, Tmax + 257, 16):
        if 1024 * C + 3 * 24576 >= best[0]:
            break
        for s1 in range(min(C - 64, 1024), C // 3 - 1, -16):
            rem = C - s1
            for s2 in range(rem - 32, (rem - 1) // 2, -16):
                s3 = rem - s2
                if not (32 <= s3 <= s2 <= s1):
                    continue
                try_sizes((s1, s2, s3))

    return best[1], best[2]


# ------------------------------------------------------------------ packing

def _pack_w1(w1_e):
    """w1_e [F, D] f32 -> [128, KO1, F] bf16."""
    return np.ascontiguousarray(
        w1_e.astype(BF16).reshape(F, KO1, P).transpose(2, 1, 0)
    )


def _pack_w2(w2_e):
    """w2_e [D, F] f32 -> [128, KO2, D] bf16."""
    return np.ascontiguousarray(
        w2_e.astype(BF16).reshape(D, KO2, P).transpose(2, 1, 0)
    )


LAST_RUN = {}


def prepare(hidden_states, router_logits, w1, w2):
    """Host-side routing + balancing + packing. Returns (nc, in_maps, meta)."""
    hidden_states = np.asarray(hidden_states)
    router_logits = np.asarray(router_logits)
    w1 = np.asarray(w1)
    w2 = np.asarray(w2)

    b, s, d = hidden_states.shape
    T = b * s
    x = hidden_states.reshape(T, d).astype(np.float32)
    assign = np.argmax(router_logits.reshape(T, E), axis=-1)

    idx = [np.nonzero(assign == e)[0] for e in range(E)]
    counts = [int(i.size) for i in idx]

    sizes, alloc = choose_scheme(counts)
    S = len(sizes)
    C = sum(sizes)
    nc = build_nc(sizes)

    # Build bins: bins[j] = list of (expert, token_idx_array) for size class j.
    bins = [[] for _ in range(S)]
    for e in range(E):
        pos = 0
        for j in range(S):
            for _ in range(alloc[e][j]):
                take = min(sizes[j], counts[e] - pos)
                bins[j].append((e, idx[e][pos : pos + take]))
                pos += take
        assert pos == counts[e], (e, pos, counts[e])
    for j in range(S):
        while len(bins[j]) < N_CORES:
            bins[j].append((0, np.zeros(0, dtype=np.int64)))

    w1_packed, w2_packed = {}, {}

    def packed(e):
        if e not in w1_packed:
            w1_packed[e] = _pack_w1(w1[e])
            w2_packed[e] = _pack_w2(w2[e])
        return w1_packed[e], w2_packed[e]

    bases = np.cumsum([0] + list(sizes))[:-1]
    in_maps = []
    core_bins = []
    for c in range(N_CORES):
        xb = np.zeros((C, D), dtype=BF16)
        slots1, slots2, cbins = [], [], []
        for j in range(S):
            e, tidx = bins[j][c]
            xb[bases[j] : bases[j] + len(tidx)] = x[tidx].astype(BF16)
            p1, p2 = packed(e)
            slots1.append(p1)
            slots2.append(p2)
            cbins.append(tidx)
        xt = np.ascontiguousarray(xb.reshape(C, KO1, P).transpose(2, 1, 0))
        if S == 1:
            in_maps.append({"xt": xt, "w1t": slots1[0], "w2t": slots2[0]})
        else:
            in_maps.append(
                {
                    "xt": xt,
                    "w1t": np.ascontiguousarray(np.stack(slots1)),
                    "w2t": np.ascontiguousarray(np.stack(slots2)),
                }
            )
        core_bins.append(cbins)

    meta = {
        "mode": "multi", "b": b, "s": s, "d": d, "T": T,
        "sizes": sizes, "C": C, "bases": bases,
        "core_bins": core_bins, "counts": counts,
    }
    return nc, in_maps, meta


def finish(results, meta):
    """Scatter per-core outputs back to token order."""
    T, d, C = meta["T"], meta["d"], meta["C"]
    bases = meta["bases"]
    out = np.zeros((T, d), dtype=np.float32)
    for c in range(N_CORES):
        yt = np.asarray(results[c]["yt"])  # [128, KO1, C] f32
        y_tok = yt.transpose(2, 1, 0).reshape(C, D)
        for j, tidx in enumerate(meta["core_bins"][c]):
            if len(tidx):
                out[tidx] = y_tok[bases[j] : bases[j] + len(tidx)]
    return out.reshape(meta["b"], meta["s"], d)


def kernel(hidden_states, router_logits, w1, w2):
    from concourse.bass_utils import run_bass_kernel_spmd

    nc, in_maps, meta = prepare(hidden_states, router_logits, w1, w2)
    res = run_bass_kernel_spmd(nc, in_maps, core_ids=list(range(N_CORES)))
    LAST_RUN["capacity"] = meta["C"]
    LAST_RUN["counts"] = meta["counts"]
    LAST_RUN["sizes"] = meta["sizes"]
    return finish(res.results, meta)


# revision 10
# speedup vs baseline: 1.0014x; 1.0014x over previous
"""Top-1 MoE (8 experts) expert-parallel kernel for Trainium2, 8 NeuronCores.

Strategy:
  - Host: argmax(router_logits) -> per-token expert id; tokens are grouped by
    expert and packed into per-core bins (the "all-to-all dispatch" happens
    host-side since we receive full inputs and return full outputs).
  - Load balance: every core runs the same program with `nsegs` token
    segments of fixed sizes (S1..Sk); each segment has its own weight slot
    (an expert id, per-core data). A DP assigns experts to the 8*nsegs bins
    so per-core capacity C = sum(sizes) is minimized (~1.6% above the
    perfect T/8 balance instead of the 23% a one-expert-per-core split
    costs with skewed routing).
  - Device (SPMD): dense 2-GEMM SiLU MLP in bf16 with fp32 PSUM
    accumulation, one weight stream per segment.
  - Host: scatter per-core outputs back to token order ("combine").

Per-core problem per segment s: x[S_s, D] @ w1[slot_s].T -> silu ->
@ w2[slot_s].T, with D=2048, F=4096.

Device layouts (partition-major so every DMA is a plain slice):
  xt  [128, 16, C]     bf16   xt[p, ko, t]     = x[t, ko*128+p]
  w1t [S, 128, 16, F]  bf16   w1t[s, p, ko, f] = w1[slot_s][f, ko*128+p]
  w2t [S, 128, 32, D]  bf16   w2t[s, p, ko, d] = w2[slot_s][d, ko*128+p]
  yt  [128, 16, C]     f32    yt[p, do, t]     = y[t, do*128+p]
"""

import numpy as np
import ml_dtypes

BF16 = ml_dtypes.bfloat16

P = 128
D = 2048
F = 4096
E = 8
N_CORES = 8
TCHUNK = 512  # max token chunk = matmul free dim (one PSUM bank of fp32)
W1B = 512     # GEMM1 weight block width (columns of F per streamed tile)
W2B = 256     # GEMM2 weight block width (columns of D per streamed tile)

KO1 = D // P  # 16 contraction tiles for GEMM1
KO2 = F // P  # 32 contraction tiles for GEMM2

_BUILD_CACHE = {}


def dedupe_ldweights(nc):
    """Delete InstLdweights that reload the stationary tile just loaded
    (identical source AP, no intervening PE-array clobber). The following
    non-self-loading InstMatmult then reuses the already-loaded weights.
    Only sync-free ldweights are removed, so all semaphore waits/updates
    are preserved. Validated bit-exact in CoreSim."""
    import concourse.mybir as mybir
    ndel = 0
    for fn in nc.m.functions:
        for blk in fn.blocks:
            last_key = None
            keep = []
            for inst in blk.instructions:
                tn = type(inst).__name__
                if tn == "InstLdweights":
                    si = inst.sync_info
                    has_sync = si is not None and (
                        len(si.on_wait) > 0 or len(si.on_update) > 0
                    )
                    key = str(inst.ins[0])
                    if (not has_sync) and key == last_key:
                        ndel += 1
                        continue
                    last_key = key
                elif tn == "InstMatmult":
                    pass  # consumes loaded weights, no clobber
                elif getattr(inst, "engine", None) == mybir.EngineType.PE:
                    last_key = None  # other PE instruction: conservative reset
                keep.append(inst)
            blk.instructions[:] = keep
    return ndel


def _chunks_of(size, base):
    """Split a segment into near-equal chunks of width <= TCHUNK."""
    n = -(-size // TCHUNK)
    out = []
    t0 = 0
    for i in range(n):
        w = (size - t0 + (n - i) - 1) // (n - i)
        out.append((base + t0, w))
        t0 += w
    return out


def build_nc(sizes, act="silu", loop_reps=None):
    """Build + compile the per-core Bass program for segment sizes `sizes`.

    Each segment uses weight slot s of the w1t/w2t inputs. loop_reps wraps
    one pass in a hardware For_i loop (for slope-based HW timing); results
    are identical since the computation is idempotent.
    """
    sizes = tuple(int(s) for s in sizes)
    key = (sizes, act, loop_reps)
    if key in _BUILD_CACHE:
        return _BUILD_CACHE[key]

    import concourse.bacc as bacc
    import concourse.mybir as mybir
    from concourse import tile

    S = len(sizes)
    C = sum(sizes)
    dt = mybir.dt
    act_fn = {
        "silu": mybir.ActivationFunctionType.Silu,
        "sigmoid": mybir.ActivationFunctionType.Sigmoid,
    }[act]
    nc = bacc.Bacc("TRN2", target_bir_lowering=False, debug=False)

    xt_d = nc.dram_tensor("xt", [P, KO1, C], dt.bfloat16, kind="ExternalInput")
    if S == 1:
        w1t_d = nc.dram_tensor("w1t", [P, KO1, F], dt.bfloat16, kind="ExternalInput")
        w2t_d = nc.dram_tensor("w2t", [P, KO2, D], dt.bfloat16, kind="ExternalInput")
        w1s = lambda s: w1t_d
        w2s = lambda s: w2t_d
    else:
        w1t_d = nc.dram_tensor("w1t", [S, P, KO1, F], dt.bfloat16, kind="ExternalInput")
        w2t_d = nc.dram_tensor("w2t", [S, P, KO2, D], dt.bfloat16, kind="ExternalInput")
        w1s = lambda s: w1t_d[s]
        w2s = lambda s: w2t_d[s]
    yt_d = nc.dram_tensor("yt", [P, KO1, C], dt.float32, kind="ExternalOutput")

    seg_chunks = []  # per segment: list of (t0, tw)
    base = 0
    for s in sizes:
        seg_chunks.append(_chunks_of(s, base))
        base += s

    N1 = F // W1B
    N2 = D // W2B

    with tile.TileContext(nc) as tc:
        with (
            tc.tile_pool(name="xpool", bufs=1) as xpool,
            tc.tile_pool(name="hpool", bufs=1) as hpool,
            tc.tile_pool(name="wpool", bufs=4) as wpool,
            tc.tile_pool(name="ypool", bufs=4) as ypool,
            tc.tile_pool(name="cpool", bufs=1) as cpool,
            tc.tile_pool(name="pspool", bufs=8, space="PSUM") as pspool,
        ):
            zbias = cpool.tile([P, 1], dt.float32)
            nc.any.memset(zbias[:], 0.0)

            x_sb = xpool.tile([P, KO1, C], dt.bfloat16)
            h_sb = hpool.tile([P, KO2, C], dt.bfloat16)

            # Load x by chunk so GEMM1 can start after the first chunk.
            # SWDGE (gpsimd) path: x never queues behind the weight prefetch
            # on the SP HWDGE ring, so the first matmul starts sooner.
            for chunks in seg_chunks:
                for (t0, tw) in chunks:
                    nc.gpsimd.dma_start(
                        x_sb[:, :, t0 : t0 + tw], xt_d[:, :, t0 : t0 + tw]
                    )

            def one_pass(rep):
                # chunk-outer / k-inner: 16-32 consecutive matmuls accumulate
                # into ONE PSUM bank. k-outer/chunk-inner (saves 1024
                # LDWEIGHTS via dedupe, pays per-matmul PSUM bank switches)
                # measured equivalent within 0.5us on HW.
                # GEMM1 + SiLU: h[f, t] = silu(sum_d w1t[d, f] * x[d, t])
                for si in range(S):
                    chunks = seg_chunks[si]
                    for mb in range(N1):
                        w1_sb = wpool.tile(
                            [P, KO1, W1B], dt.bfloat16, tag="w",
                            name=f"w1_{rep}_{si}_{mb}",
                        )
                        nc.sync.dma_start(
                            w1_sb[:], w1s(si)[:, :, mb * W1B : (mb + 1) * W1B]
                        )
                        for ms in range(W1B // P):
                            fo = mb * (W1B // P) + ms
                            for ci, (t0, tw) in enumerate(chunks):
                                ps = pspool.tile(
                                    [P, TCHUNK], dt.float32, tag="ps",
                                    name=f"ps1_{rep}_{si}_{mb}_{ms}_{ci}",
                                )
                                for k in range(KO1):
                                    nc.tensor.matmul(
                                        ps[:, :tw],
                                        w1_sb[:, k, ms * P : (ms + 1) * P],
                                        x_sb[:, k, t0 : t0 + tw],
                                        start=(k == 0),
                                        stop=(k == KO1 - 1),
                                    )
                                nc.scalar.activation(
                                    h_sb[:, fo, t0 : t0 + tw],
                                    ps[:, :tw],
                                    act_fn,
                                    bias=zbias[:],
                                )

                # GEMM2: y[d, t] = sum_f w2t[f, d] * h[f, t]
                for si in range(S):
                    chunks = seg_chunks[si]
                    for db in range(N2):
                        w2_sb = wpool.tile(
                            [P, KO2, W2B], dt.bfloat16, tag="w",
                            name=f"w2_{rep}_{si}_{db}",
                        )
                        nc.sync.dma_start(
                            w2_sb[:], w2s(si)[:, :, db * W2B : (db + 1) * W2B]
                        )
                        for ds in range(W2B // P):
                            do = db * (W2B // P) + ds
                            for ci, (t0, tw) in enumerate(chunks):
                                ps = pspool.tile(
                                    [P, TCHUNK], dt.float32, tag="ps",
                                    name=f"ps2_{rep}_{si}_{db}_{ds}_{ci}",
                                )
                                for k in range(KO2):
                                    nc.tensor.matmul(
                                        ps[:, :tw],
                                        w2_sb[:, k, ds * P : (ds + 1) * P],
                                        h_sb[:, k, t0 : t0 + tw],
                                        start=(k == 0),
                                        stop=(k == KO2 - 1),
                                    )
                                y_sb = ypool.tile(
                                    [P, TCHUNK], dt.float32, tag="y",
                                    name=f"y_{rep}_{si}_{db}_{ds}_{ci}",
                                )
                                nc.vector.tensor_copy(y_sb[:, :tw], ps[:, :tw])
                                # y stores go through the ACT HWDGE ring so
                                # they never queue ahead of weight prefetch on
                                # the SP ring (HWDGE is FIFO per engine).
                                nc.scalar.dma_start(
                                    yt_d[:, do, t0 : t0 + tw], y_sb[:, :tw]
                                )

            if loop_reps is not None and loop_reps > 1:
                with tc.For_i(0, loop_reps, 1):
                    one_pass(0)
            else:
                one_pass(0)

    nc.compile()
    dedupe_ldweights(nc)
    _BUILD_CACHE[key] = nc
    return nc


# ---------------------------------------------------------------- balancing

def _min_cover_opts(c, sizes, max_bins):
    """All minimal k-vectors with sum_j k_j*S_j >= c."""
    import itertools
    if c == 0:
        return [tuple([0] * len(sizes))]
    opts = []
    maxk = [min(max_bins, -(-c // s)) for s in sizes]
    for ks in itertools.product(*[range(k + 1) for k in maxk]):
        cap = sum(k * s for k, s in zip(ks, sizes))
        if cap < c:
            continue
        if any(k > 0 and cap - s >= c for k, s in zip(ks, sizes)):
            continue
        opts.append(ks)
    return opts


def _solve_alloc(counts, sizes, nbins=N_CORES):
    """DP: assign each expert k_j bins of size S_j (sum_j k_j*S_j >= count),
    using at most `nbins` bins of each size overall. Returns per-expert
    k-vectors or None."""
    ns = len(sizes)
    levels = [{tuple([0] * ns): None}]
    for c in counts:
        opts = _min_cover_opts(c, sizes, nbins)
        new = {}
        for st in levels[-1]:
            for ks in opts:
                nst = tuple(u + k for u, k in zip(st, ks))
                if all(u <= nbins for u in nst) and nst not in new:
                    new[nst] = (st, ks)
        if not new:
            return None
        levels.append(new)
    state = next(iter(levels[-1]))
    alloc = []
    for lev in range(len(counts), 0, -1):
        prev, ks = levels[lev][state]
        alloc.append(ks)
        state = prev
    alloc.reverse()
    return alloc


def _pass_cost(sizes):
    """Relative per-pass cost model (cycles): streamed rows + per-segment
    weight restream overhead + per-chunk overhead. Narrow chunks stream
    slightly worse on HW (measured ~5% at width 128), so penalize them."""
    C = sum(sizes)
    nseg = len(sizes)
    cost = 1024 * C + 24576 * nseg
    for s in sizes:
        for _, w in _chunks_of(s, 0):
            cost += 8192 + 24 * max(0, 224 - w)
    return cost


def choose_scheme(counts):
    """Pick segment sizes minimizing _pass_cost subject to DP feasibility."""
    counts = list(counts)
    best = (float("inf"), (max(max(counts), P),), None)

    # 1 segment
    s1 = max(max(counts), P)
    alloc = _solve_alloc(counts, (s1,))
    if alloc is not None:
        best = min(best, (_pass_cost((s1,)), (s1,), alloc))

    # 2 segments: S1 >= S2, coarse then fine grid
    def try_sizes(sizes):
        nonlocal best
        if sizes[-1] < 32:
            return
        cost = _pass_cost(sizes)
        if cost >= best[0]:
            return
        alloc = _solve_alloc(counts, sizes)
        if alloc is not None:
            best = min(best, (cost, sizes, alloc))

    Tmax = max(max(counts), -(-sum(counts) // N_CORES))
    for C in range(-(-sum(counts) // (N_CORES * 16)) * 16, Tmax + 257, 16):
        for s2 in range(32, C // 2 + 1, 16):
            try_sizes((C - s2, s2))
        if best[2] is not None and best[0] <= 1024 * (C + 64) + 2 * 24576:
            break

    # 3 segments (s1-major grid so 512-wide bulk segments are explored)
    for C in range(-(-sum(counts) // (N_CORES * 16)) * 16, Tmax + 257, 16):
        if 1024 * C + 3 * 24576 >= best[0]:
            break
        for s1 in range(min(C - 64, 1024), C // 3 - 1, -16):
            rem = C - s1
            for s2 in range(rem - 32, (rem - 1) // 2, -16):
                s3 = rem - s2
                if not (32 <= s3 <= s2 <= s1):
                    continue
                try_sizes((s1, s2, s3))

    return best[1], best[2]


# ------------------------------------------------------------------ packing

def _pack_w1(w1_e):
    """w1_e [F, D] f32 -> [128, KO1, F] bf16."""
    return np.ascontiguousarray(
        w1_e.astype(BF16).reshape(F, KO1, P).transpose(2, 1, 0)
    )


def _pack_w2(w2_e):
    """w2_e [D, F] f32 -> [128, KO2, D] bf16."""
    return np.ascontiguousarray(
        w2_e.astype(BF16).reshape(D, KO2, P).transpose(2, 1, 0)
    )


LAST_RUN = {}


def prepare(hidden_states, router_logits, w1, w2):
    """Host-side routing + balancing + packing. Returns (nc, in_maps, meta)."""
    hidden_states = np.asarray(hidden_states)
    router_logits = np.asarray(router_logits)
    w1 = np.asarray(w1)
    w2 = np.asarray(w2)

    b, s, d = hidden_states.shape
    T = b * s
    x = hidden_states.reshape(T, d).astype(np.float32)
    assign = np.argmax(router_logits.reshape(T, E), axis=-1)

    idx = [np.nonzero(assign == e)[0] for e in range(E)]
    counts = [int(i.size) for i in idx]

    sizes, alloc = choose_scheme(counts)
    S = len(sizes)
    C = sum(sizes)
    nc = build_nc(sizes)

    # Build bins: bins[j] = list of (expert, token_idx_array) for size class j.
    bins = [[] for _ in range(S)]
    for e in range(E):
        pos = 0
        for j in range(S):
            for _ in range(alloc[e][j]):
                take = min(sizes[j], counts[e] - pos)
                bins[j].append((e, idx[e][pos : pos + take]))
                pos += take
        assert pos == counts[e], (e, pos, counts[e])
    for j in range(S):
        while len(bins[j]) < N_CORES:
            bins[j].append((0, np.zeros(0, dtype=np.int64)))

    w1_packed, w2_packed = {}, {}

    def packed(e):
        if e not in w1_packed:
            w1_packed[e] = _pack_w1(w1[e])
            w2_packed[e] = _pack_w2(w2[e])
        return w1_packed[e], w2_packed[e]

    bases = np.cumsum([0] + list(sizes))[:-1]
    in_maps = []
    core_bins = []
    for c in range(N_CORES):
        xb = np.zeros((C, D), dtype=BF16)
        slots1, slots2, cbins = [], [], []
        for j in range(S):
            e, tidx = bins[j][c]
            xb[bases[j] : bases[j] + len(tidx)] = x[tidx].astype(BF16)
            p1, p2 = packed(e)
            slots1.append(p1)
            slots2.append(p2)
            cbins.append(tidx)
        xt = np.ascontiguousarray(xb.reshape(C, KO1, P).transpose(2, 1, 0))
        if S == 1:
            in_maps.append({"xt": xt, "w1t": slots1[0], "w2t": slots2[0]})
        else:
            in_maps.append(
                {
                    "xt": xt,
                    "w1t": np.ascontiguousarray(np.stack(slots1)),
                    "w2t": np.ascontiguousarray(np.stack(slots2)),
                }
            )
        core_bins.append(cbins)

    meta = {
        "mode": "multi", "b": b, "s": s, "d": d, "T": T,
        "sizes": sizes, "C": C, "bases": bases,
        "core_bins": core_bins, "counts": counts,
    }
    return nc, in_maps, meta


def finish(results, meta):
    """Scatter per-core outputs back to token order."""
    T, d, C = meta["T"], meta["d"], meta["C"]
    bases = meta["bases"]
    out = np.zeros((T, d), dtype=np.float32)
    for c in range(N_CORES):
        yt = np.asarray(results[c]["yt"])  # [128, KO1, C] f32
        y_tok = yt.transpose(2, 1, 0).reshape(C, D)
        for j, tidx in enumerate(meta["core_bins"][c]):
            if len(tidx):
                out[tidx] = y_tok[bases[j] : bases[j] + len(tidx)]
    return out.reshape(meta["b"], meta["s"], d)


def kernel(hidden_states, router_logits, w1, w2):
    from concourse.bass_utils import run_bass_kernel_spmd

    nc, in_maps, meta = prepare(hidden_states, router_logits, w1, w2)
    res = run_bass_kernel_spmd(nc, in_maps, core_ids=list(range(N_CORES)))
    LAST_RUN["capacity"] = meta["C"]
    LAST_RUN["counts"] = meta["counts"]
    LAST_RUN["sizes"] = meta["sizes"]
    return finish(res.results, meta)
